# revision 36
# baseline (speedup 1.0000x reference)
"""Trainium2 Bass kernel for nn_AttnInteractionLayer_16982300688923.

Math: the reference's einsum 'rfdh,rfoh->rfoh' contracts alpha over its own
softmax axis, which sums to exactly 1 — so the whole Q/K/softmax pipeline
collapses to out == vals.  The remaining computation is

    y   = x @ (m*W_v + (1-m)*W_r)          m = sigmoid(mix)  (host-folded)
    y   = relu(y)
    out = (y - mean(y)) * rsqrt(var(y)+eps) * gamma + beta    (LN over last dim)

Sharding: data-parallel over R across 8 cores; weights replicated.

bf16 edition: the kernel is DMA-bound at fp32 (~330 GB/s/core, 50MB/core), so
x is cast to bf16 on the host (pre-transposed so the contraction dim lands on
SBUF partitions) and the output is written bf16 and upcast on the host —
halving both directions.  bf16 matmul runs at the same 1 cycle/row as fp32r.

Per-core device pipeline (rows = R/8 * F = 16384, bf16, 16 slabs of 8
128-row blocks), emitted as a 3-stage software pipeline so no engine's
in-order queue ever holds an instruction whose deps are still in flight:
  sync-queue DMA x^T slab -> 16 bf16 matmuls accumulate 8x Y[128,512] in
  PSUM (two 4-bank tiles) -> ACT relu PSUM->SBUF bf16 (2048-wide) ->
  DVE bn_stats+bn_aggr per block || ACT sqrt(var+eps) + DVE reciprocal
  ([P,8] batched, one slab behind) || apply y*rstd+nm per block on
  GPSIMD/ACT (DVE for the drain slabs), two slabs behind -> sync-queue
  DMA out per slab.  A PE warm-up burst releases the HAM clock gate
  during the first input DMA.
Uniform gamma/beta fold into the scalars; per-feature gamma/beta get two
extra per-block passes (general path, not hit by the harness inputs).
"""

import os as _os

import numpy as np

R, F, D_IN = 2048, 64, 256
OH = 512  # output_dim * num_head
N_CORES = 8
ROWS_PER_CORE = (R // N_CORES) * F  # 16384
P = 128
BLOCKS = ROWS_PER_CORE // P  # 128
SLAB = 8  # 128-row blocks per slab
N_SLABS = BLOCKS // SLAB  # 16
EPS = 1e-5

# apply-pass engine per block-in-slab (8 chars), out = y*rstd + nm form with
# plain [P,8] scalar tiles (strided scalar APs are catastrophically slow):
#   G = GPSIMD tensor_scalar   A = ACT Identity scale/bias   D = DVE tensor_scalar
# Steady-state slabs keep applies OFF the pace-setting DVE; the last two
# slabs go all-DVE because during pipeline drain DVE is the idle engine.
APPLY_STEADY = _os.environ.get("K_APPLY", "GGGGGGGA")
# during drain the last two slabs' applies go to DVE, which is otherwise
# idle once its stats work is done
APPLY_TAIL1 = _os.environ.get("K_APPLY_TAIL1", "DDDDDDDD")
APPLY_TAIL0 = _os.environ.get("K_APPLY_TAIL0", "DDDDDDDD")

_prog_cache = {}


def _build(affine_mode, g_u, b_u):
    """affine_mode: 'none' (uniform gamma/beta folded into scalars g_u/b_u)
    or 'full' (per-feature gamma/beta tensors applied on device)."""
    from contextlib import ExitStack

    import concourse.bass as bass
    import concourse.mybir as mybir
    import concourse.tile as tile
    from concourse import bacc

    f32 = mybir.dt.float32
    bf16 = mybir.dt.bfloat16
    AF = mybir.ActivationFunctionType
    OP = mybir.AluOpType

    nc = bacc.Bacc(trn_type="TRN2", target_bir_lowering=False)
    # Host-permuted input: [p, s, ko, r] so each partition reads one
    # contiguous 4KB run per slab.
    xt = nc.dram_tensor(
        "xt", [P, N_SLABS, 2, SLAB * P], bf16, kind="ExternalInput"
    )
    wc = nc.dram_tensor("w", [D_IN, OH], bf16, kind="ExternalInput")
    if affine_mode == "full":
        gam = nc.dram_tensor("gamma", [OH], f32, kind="ExternalInput")
        bet = nc.dram_tensor("beta", [OH], f32, kind="ExternalInput")
    # Host-unpermuted output: [s, p, b, n] so each partition writes one
    # contiguous 8KB run per slab.
    out = nc.dram_tensor(
        "out", [N_SLABS, P, SLAB, OH], bf16, kind="ExternalOutput"
    )

    with ExitStack() as ctx:
        tc = ctx.enter_context(tile.TileContext(nc))
        const = ctx.enter_context(tc.tile_pool(name="const", bufs=1))
        xin = ctx.enter_context(tc.tile_pool(name="xin", bufs=4))
        xin4 = ctx.enter_context(tc.tile_pool(name="xin4", bufs=3))
        psy = ctx.enter_context(tc.tile_pool(name="psy", bufs=2, space="PSUM"))
        yrp = ctx.enter_context(tc.tile_pool(name="yrp", bufs=4))
        yrp4 = ctx.enter_context(tc.tile_pool(name="yrp4", bufs=3))
        stp = ctx.enter_context(tc.tile_pool(name="stp", bufs=10))
        outp = ctx.enter_context(tc.tile_pool(name="outp", bufs=3))
        outp4 = ctx.enter_context(tc.tile_pool(name="outp4", bufs=3))

        w_sb = const.tile([P, 2, OH], bf16)
        nc.sync.dma_start(w_sb, wc.rearrange("(ko p) n -> p ko n", p=P))
        eps_sb = const.tile([P, 1], f32)
        nc.vector.memset(eps_sb, EPS)
        # PE warm-up operand: ~3.4us of matmul activity releases the HAM
        # clock gate (1.2 -> 2.4 GHz) while the first input DMA is in flight
        warm = const.tile([P, P], bf16)
        nc.vector.memset(warm, 0.0)
        if affine_mode == "full":
            g_sb = const.tile([P, OH], f32)
            b_sb = const.tile([P, OH], f32)
            nc.sync.dma_start(
                g_sb, bass.AP(tensor=gam.tensor, offset=gam.offset, ap=[[0, P], *gam.ap])
            )
            nc.sync.dma_start(
                b_sb, bass.AP(tensor=bet.tensor, offset=bet.offset, ap=[[0, P], *bet.ap])
            )

        # Software-pipelined emission over variable-size chunks, 3 stages
        # deep: front(c) computes matmul/relu/stats for chunk c; LN scalar
        # math runs one chunk behind; applies + out-DMA two chunks behind.
        # This keeps every engine's in-order queue free of instructions
        # whose deps are still in flight (the naive order lost ~35us to ACT
        # stalling on DVE stats each slab).  The first and last two chunks
        # are 4-block half-slabs so the fill/drain latency chains
        # (DMA->matmul->relu->stats->LN->apply->DMA) are half as long.
        chunks = (
            [(0, 4), (4, 4)]
            + [(b, 8) for b in range(8, BLOCKS - 8, 8)]
            + [(BLOCKS - 8, 4), (BLOCKS - 4, 4)]
        )
        NC = len(chunks)
        state = {}

        def front(c):
            b0, nb = chunks[c]
            s, off = b0 // SLAB, b0 % SLAB
            xt_sl = (xin if nb == SLAB else xin4).tile([P, 2, nb * P], bf16)
            # with pipelined emission in(c) is enqueued ahead of out(c-2),
            # so the Sync FIFO never starves the input prefetch
            nc.sync.dma_start(xt_sl, xt[:, s, :, off * P : (off + nb) * P])
            yr_sl = (yrp if nb == SLAB else yrp4).tile([P, nb, OH], bf16)
            # [P, 2, nb]: means packed at [:,0,:], vars at [:,1,:] — the
            # downstream [P,nb] reads must be unstrided (strided TensorScalar
            # reads fall off the DVE fast path)
            mv = stp.tile([P, 2, SLAB], f32, tag="mv")
            for h in range(nb // 4):
                py4 = psy.tile([P, 4, OH], f32)
                if c == 0 and h == 0:
                    # warm-up matmuls into the first real PSUM tile: ~3.4us
                    # of PE activity releases the HAM clock gate while the
                    # first input DMA is in flight (region re-initialized by
                    # the real start=True matmul)
                    for _ in range(32):
                        nc.tensor.matmul(
                            py4[:, 0, :P], warm, warm, start=True, stop=True
                        )
                for k in range(4):
                    b = h * 4 + k
                    nc.tensor.matmul(
                        py4[:, k], xt_sl[:, 0, b * P : (b + 1) * P], w_sb[:, 0],
                        start=True, stop=False,
                    )
                    nc.tensor.matmul(
                        py4[:, k], xt_sl[:, 1, b * P : (b + 1) * P], w_sb[:, 1],
                        start=False, stop=True,
                    )
                # one relu covers 4 blocks (4 PSUM banks -> 2048 wide)
                nc.scalar.activation(yr_sl[:, h * 4 : h * 4 + 4], py4, AF.Relu)
                for k in range(4):
                    b = h * 4 + k
                    st = stp.tile([P, 6], f32, tag="bnst")
                    nc.vector.bn_stats(st, yr_sl[:, b])
                    nc.vector.bn_aggr(mv[:, :, b], st)
            state[c] = {"yr": yr_sl, "mv": mv}

        def ln_math(c):
            b0, nb = chunks[c]
            st_c = state[c]
            mv = st_c["mv"]
            rs8 = stp.tile([P, SLAB], f32, tag="rs")
            nm8 = stp.tile([P, SLAB], f32, tag="nm")
            # rstd = 1/sqrt(var+eps) (*g_u); nm = -mu*rstd (+b_u).  The
            # approx reciprocal (~51 ULP) is 5x cheaper than the iterative
            # one and its input is >= sqrt(eps), far from any edge case.
            nc.scalar.activation(rs8[:, :nb], mv[:, 1, :nb], AF.Sqrt, bias=eps_sb)
            nc.vector.reciprocal_approx_fast(rs8[:, :nb], rs8[:, :nb])
            if affine_mode == "none" and g_u != 1.0:
                nc.vector.tensor_scalar_mul(rs8[:, :nb], rs8[:, :nb], float(g_u))
            nc.vector.tensor_scalar_mul(nm8[:, :nb], mv[:, 0, :nb], -1.0)
            nc.vector.tensor_tensor(nm8[:, :nb], nm8[:, :nb], rs8[:, :nb], OP.mult)
            if affine_mode == "none" and b_u != 0.0:
                nc.vector.tensor_scalar_add(nm8[:, :nb], nm8[:, :nb], float(b_u))
            st_c["rs8"] = rs8
            st_c["nm8"] = nm8

        def back(c):
            b0, nb = chunks[c]
            s, off = b0 // SLAB, b0 % SLAB
            st_c = state.pop(c)
            yr_sl, rs8, nm8 = st_c["yr"], st_c["rs8"], st_c["nm8"]
            if c >= NC - 2:
                engines = APPLY_TAIL0  # drain: DVE is the idle engine
            else:
                engines = APPLY_STEADY
            ob = (outp if nb == SLAB else outp4).tile([P, nb, OH], bf16)
            for j in range(nb):
                rs_ap = rs8[:, j : j + 1]
                nm_ap = nm8[:, j : j + 1]
                # out = y*rstd + nm  (nm = -mu*rstd)
                eng = engines[j]
                if eng == "D":
                    nc.vector.tensor_scalar(
                        ob[:, j], yr_sl[:, j], rs_ap, nm_ap, OP.mult, OP.add
                    )
                elif eng == "A":
                    nc.scalar.activation(
                        ob[:, j], yr_sl[:, j], AF.Identity, bias=nm_ap, scale=rs_ap
                    )
                else:
                    nc.gpsimd.tensor_scalar(
                        ob[:, j], yr_sl[:, j], rs_ap, nm_ap, OP.mult, OP.add
                    )
                if affine_mode == "full":
                    nc.vector.tensor_tensor(ob[:, j], ob[:, j], g_sb, OP.mult)
                    nc.gpsimd.tensor_tensor(ob[:, j], ob[:, j], b_sb, OP.add)
            nc.sync.dma_start(out[s, :, off : off + nb], ob)

        for c in range(NC + 2):
            if c < NC:
                front(c)
            if 0 <= c - 1 < NC:
                ln_math(c - 1)
            if c >= 2:
                back(c - 2)
    nc.finalize()
    return nc


def _get_prog(affine_mode, g_u, b_u):
    key = (affine_mode, g_u, b_u)
    if key not in _prog_cache:
        _prog_cache[key] = _build(affine_mode, g_u, b_u)
    return _prog_cache[key]


def _prepare(x, W_q, W_k, W_v, W_r, mix, gamma, beta):
    import ml_dtypes

    bf16 = ml_dtypes.bfloat16
    x = np.asarray(x, dtype=np.float32)
    W_v = np.asarray(W_v, dtype=np.float32)
    W_r = np.asarray(W_r, dtype=np.float32)
    gamma = np.asarray(gamma, dtype=np.float32)
    beta = np.asarray(beta, dtype=np.float32)
    m = 1.0 / (1.0 + np.exp(-float(np.asarray(mix).reshape(-1)[0])))
    wc = np.ascontiguousarray((m * W_v + (1.0 - m) * W_r).astype(bf16))

    if np.all(gamma == gamma.flat[0]) and np.all(beta == beta.flat[0]):
        affine_mode, g_u, b_u = "none", float(gamma.flat[0]), float(beta.flat[0])
    else:
        affine_mode, g_u, b_u = "full", 1.0, 0.0

    x_flat = x.reshape(R * F, D_IN).astype(bf16)
    in_maps = []
    for c in range(N_CORES):
        shard = x_flat[c * ROWS_PER_CORE : (c + 1) * ROWS_PER_CORE]
        # [p, s, ko, r] layout: contiguous 4KB per (partition, slab)
        xt_h = np.ascontiguousarray(
            shard.reshape(N_SLABS, SLAB * P, 2, P).transpose(3, 0, 2, 1)
        )
        im = {"xt": xt_h, "w": wc}
        if affine_mode == "full":
            im["gamma"] = gamma
            im["beta"] = beta
        in_maps.append(im)
    return in_maps, affine_mode, g_u, b_u


def _unpermute_out(arr):
    # [s, p, b, n] -> rows ordered (s, b, p)
    return arr.transpose(0, 2, 1, 3).reshape(ROWS_PER_CORE, OH)


def run(trace=False, **inputs):
    """Internal entry: returns (output, BassKernelResults)."""
    from concourse.bass_utils import run_bass_kernel_spmd

    in_maps, affine_mode, g_u, b_u = _prepare(**inputs)
    nc = _get_prog(affine_mode, g_u, b_u)
    res = run_bass_kernel_spmd(nc, in_maps, core_ids=list(range(N_CORES)), trace=trace)
    parts = [
        _unpermute_out(np.asarray(r["out"], dtype=np.float32)).reshape(
            R // N_CORES, F, OH
        )
        for r in res.results
    ]
    return np.concatenate(parts, axis=0), res


def kernel(**inputs):
    out, _ = run(trace=False, **inputs)
    return out


# revision 39
# speedup vs baseline: 1.1726x; 1.1726x over previous
"""Trainium2 Bass kernel for nn_AttnInteractionLayer_16982300688923.

Math: the reference's einsum 'rfdh,rfoh->rfoh' contracts alpha over its own
softmax axis, which sums to exactly 1 — so the whole Q/K/softmax pipeline
collapses to out == vals.  The remaining computation is

    y   = x @ (m*W_v + (1-m)*W_r)          m = sigmoid(mix)  (host-folded)
    y   = relu(y)
    out = (y - mean(y)) * rsqrt(var(y)+eps) * gamma + beta    (LN over last dim)

Sharding: data-parallel over R across 8 cores; weights replicated.

bf16 edition: the kernel is DMA-bound at fp32 (~330 GB/s/core, 50MB/core), so
x is cast to bf16 on the host (pre-transposed so the contraction dim lands on
SBUF partitions) and the output is written bf16 and upcast on the host —
halving both directions.  bf16 matmul runs at the same 1 cycle/row as fp32r.

Per-core device pipeline (rows = R/8 * F = 16384, bf16, 16 slabs of 8
128-row blocks), emitted as a 3-stage software pipeline so no engine's
in-order queue ever holds an instruction whose deps are still in flight:
  sync-queue DMA x^T slab -> 16 bf16 matmuls accumulate 8x Y[128,512] in
  PSUM (two 4-bank tiles) -> ACT relu PSUM->SBUF bf16 (2048-wide) ->
  DVE bn_stats+bn_aggr per block || ACT sqrt(var+eps) + DVE reciprocal
  ([P,8] batched, one slab behind) || apply y*rstd+nm per block on
  GPSIMD/ACT (DVE for the drain slabs), two slabs behind -> sync-queue
  DMA out per slab.  A PE warm-up burst releases the HAM clock gate
  during the first input DMA.
Uniform gamma/beta fold into the scalars; per-feature gamma/beta get two
extra per-block passes (general path, not hit by the harness inputs).
"""

import os as _os

import numpy as np

R, F, D_IN = 2048, 64, 256
OH = 512  # output_dim * num_head
N_CORES = 8
ROWS_PER_CORE = (R // N_CORES) * F  # 16384
P = 128
BLOCKS = ROWS_PER_CORE // P  # 128
SLAB = 8  # 128-row blocks per slab
N_SLABS = BLOCKS // SLAB  # 16
EPS = 1e-5

# apply-pass engine per block-in-slab (8 chars), out = y*rstd + nm form with
# plain [P,8] scalar tiles (strided scalar APs are catastrophically slow):
#   G = GPSIMD tensor_scalar   A = ACT Identity scale/bias   D = DVE tensor_scalar
# Steady-state slabs keep applies OFF the pace-setting DVE; the last two
# slabs go all-DVE because during pipeline drain DVE is the idle engine.
APPLY_STEADY = _os.environ.get("K_APPLY", "GGGGGGGA")
# during drain the last two slabs' applies go to DVE, which is otherwise
# idle once its stats work is done
APPLY_TAIL1 = _os.environ.get("K_APPLY_TAIL1", "DDDDDDDD")
APPLY_TAIL0 = _os.environ.get("K_APPLY_TAIL0", "DDDDDDDD")

_prog_cache = {}


def _build(affine_mode, g_u, b_u):
    """affine_mode: 'none' (uniform gamma/beta folded into scalars g_u/b_u)
    or 'full' (per-feature gamma/beta tensors applied on device)."""
    from contextlib import ExitStack

    import concourse.bass as bass
    import concourse.mybir as mybir
    import concourse.tile as tile
    from concourse import bacc

    f32 = mybir.dt.float32
    bf16 = mybir.dt.bfloat16
    AF = mybir.ActivationFunctionType
    OP = mybir.AluOpType

    nc = bacc.Bacc(trn_type="TRN2", target_bir_lowering=False)
    # Host-permuted input: [p, s, ko, r] so each partition reads one
    # contiguous 4KB run per slab.
    xt = nc.dram_tensor(
        "xt", [P, N_SLABS, 2, SLAB * P], bf16, kind="ExternalInput"
    )
    wc = nc.dram_tensor("w", [D_IN, OH], bf16, kind="ExternalInput")
    if affine_mode == "full":
        gam = nc.dram_tensor("gamma", [OH], f32, kind="ExternalInput")
        bet = nc.dram_tensor("beta", [OH], f32, kind="ExternalInput")
    # Host-unpermuted output: [s, p, b, n] so each partition writes one
    # contiguous 8KB run per slab.
    out = nc.dram_tensor(
        "out", [N_SLABS, P, SLAB, OH], bf16, kind="ExternalOutput"
    )

    with ExitStack() as ctx:
        tc = ctx.enter_context(tile.TileContext(nc))
        const = ctx.enter_context(tc.tile_pool(name="const", bufs=1))
        xin = ctx.enter_context(tc.tile_pool(name="xin", bufs=4))
        xin4 = ctx.enter_context(tc.tile_pool(name="xin4", bufs=3))
        psy = ctx.enter_context(tc.tile_pool(name="psy", bufs=2, space="PSUM"))
        yrp = ctx.enter_context(tc.tile_pool(name="yrp", bufs=4))
        yrp4 = ctx.enter_context(tc.tile_pool(name="yrp4", bufs=3))
        stp = ctx.enter_context(tc.tile_pool(name="stp", bufs=10))
        outp = ctx.enter_context(tc.tile_pool(name="outp", bufs=3))
        outp4 = ctx.enter_context(tc.tile_pool(name="outp4", bufs=3))

        w_sb = const.tile([P, 2, OH], bf16)
        nc.sync.dma_start(w_sb, wc.rearrange("(ko p) n -> p ko n", p=P))
        eps_sb = const.tile([P, 1], f32)
        nc.vector.memset(eps_sb, EPS)
        # PE warm-up operand: ~3.4us of matmul activity releases the HAM
        # clock gate (1.2 -> 2.4 GHz) while the first input DMA is in flight
        warm = const.tile([P, P], bf16)
        nc.vector.memset(warm, 0.0)
        if affine_mode == "full":
            g_sb = const.tile([P, OH], f32)
            b_sb = const.tile([P, OH], f32)
            nc.sync.dma_start(
                g_sb, bass.AP(tensor=gam.tensor, offset=gam.offset, ap=[[0, P], *gam.ap])
            )
            nc.sync.dma_start(
                b_sb, bass.AP(tensor=bet.tensor, offset=bet.offset, ap=[[0, P], *bet.ap])
            )

        # Software-pipelined emission over variable-size chunks, 3 stages
        # deep: front(c) computes matmul/relu/stats for chunk c; LN scalar
        # math runs one chunk behind; applies + out-DMA two chunks behind.
        # This keeps every engine's in-order queue free of instructions
        # whose deps are still in flight (the naive order lost ~35us to ACT
        # stalling on DVE stats each slab).  The first and last two chunks
        # are 4-block half-slabs so the fill/drain latency chains
        # (DMA->matmul->relu->stats->LN->apply->DMA) are half as long.
        # 2-block chunks at the very edges halve the first/last dependency
        # chains again; 4-block chunks bridge to the 8-block steady state.
        chunks = (
            [(0, 2), (2, 2), (4, 4)]
            + [(b, 8) for b in range(8, BLOCKS - 16, 8)]
            + [(112, 4), (116, 4), (120, 4), (124, 2), (126, 2)]
        )
        NC = len(chunks)
        state = {}

        def front(c):
            b0, nb = chunks[c]
            s, off = divmod(b0, SLAB)
            big = nb == SLAB
            # small chunks borrow 4-block pool tiles and use a slice (PSUM
            # and SBUF pools stay homogeneous)
            xt_t = (xin if big else xin4).tile([P, 2, (8 if big else 4) * P], bf16)
            xt_sl = xt_t[:, :, : nb * P]
            # with pipelined emission in(c) is enqueued ahead of out(c-2),
            # so the Sync FIFO never starves the input prefetch
            nc.sync.dma_start(xt_sl, xt[:, s, :, off * P : (off + nb) * P])
            yr_sl = (yrp if big else yrp4).tile([P, 8 if big else 4, OH], bf16)
            # [P, 2, nb]: means packed at [:,0,:], vars at [:,1,:] — the
            # downstream [P,nb] reads must be unstrided (strided TensorScalar
            # reads fall off the DVE fast path)
            mv = stp.tile([P, 2, SLAB], f32, tag="mv")
            for h in range((nb + 3) // 4):
                g = min(4, nb - 4 * h)
                py4 = psy.tile([P, 4, OH], f32)
                if c == 0 and h == 0:
                    # warm-up matmuls into the first real PSUM tile: ~3.4us
                    # of PE activity releases the HAM clock gate while the
                    # first input DMA is in flight (region re-initialized by
                    # the real start=True matmul)
                    for _ in range(32):
                        nc.tensor.matmul(
                            py4[:, 0, :P], warm, warm, start=True, stop=True
                        )
                for k in range(g):
                    b = h * 4 + k
                    nc.tensor.matmul(
                        py4[:, k], xt_sl[:, 0, b * P : (b + 1) * P], w_sb[:, 0],
                        start=True, stop=False,
                    )
                    nc.tensor.matmul(
                        py4[:, k], xt_sl[:, 1, b * P : (b + 1) * P], w_sb[:, 1],
                        start=False, stop=True,
                    )
                # one relu covers the whole group (g PSUM banks -> g*512 wide)
                nc.scalar.activation(
                    yr_sl[:, h * 4 : h * 4 + g], py4[:, :g], AF.Relu
                )
                for k in range(g):
                    b = h * 4 + k
                    st = stp.tile([P, 6], f32, tag="bnst")
                    nc.vector.bn_stats(st, yr_sl[:, b])
                    nc.vector.bn_aggr(mv[:, :, b], st)
            state[c] = {"yr": yr_sl, "mv": mv}

        def ln_math(c):
            b0, nb = chunks[c]
            st_c = state[c]
            mv = st_c["mv"]
            rs8 = stp.tile([P, SLAB], f32, tag="rs")
            nm8 = stp.tile([P, SLAB], f32, tag="nm")
            # rstd = 1/sqrt(var+eps) (*g_u); nm = -mu*rstd (+b_u).  The
            # approx reciprocal (~51 ULP) is 5x cheaper than the iterative
            # one and its input is >= sqrt(eps), far from any edge case.
            nc.scalar.activation(rs8[:, :nb], mv[:, 1, :nb], AF.Sqrt, bias=eps_sb)
            nc.vector.reciprocal_approx_fast(rs8[:, :nb], rs8[:, :nb])
            if affine_mode == "none" and g_u != 1.0:
                nc.vector.tensor_scalar_mul(rs8[:, :nb], rs8[:, :nb], float(g_u))
            nc.vector.tensor_scalar_mul(nm8[:, :nb], mv[:, 0, :nb], -1.0)
            nc.vector.tensor_tensor(nm8[:, :nb], nm8[:, :nb], rs8[:, :nb], OP.mult)
            if affine_mode == "none" and b_u != 0.0:
                nc.vector.tensor_scalar_add(nm8[:, :nb], nm8[:, :nb], float(b_u))
            st_c["rs8"] = rs8
            st_c["nm8"] = nm8

        def back(c):
            b0, nb = chunks[c]
            s, off = divmod(b0, SLAB)
            st_c = state.pop(c)
            yr_sl, rs8, nm8 = st_c["yr"], st_c["rs8"], st_c["nm8"]
            if c >= NC - 3:
                engines = APPLY_TAIL0  # drain: DVE is the idle engine
            else:
                engines = APPLY_STEADY
            big = nb == SLAB
            ob_t = (outp if big else outp4).tile([P, 8 if big else 4, OH], bf16)
            ob = ob_t[:, :nb]
            for j in range(nb):
                rs_ap = rs8[:, j : j + 1]
                nm_ap = nm8[:, j : j + 1]
                # out = y*rstd + nm  (nm = -mu*rstd)
                eng = engines[j]
                if eng == "D":
                    nc.vector.tensor_scalar(
                        ob[:, j], yr_sl[:, j], rs_ap, nm_ap, OP.mult, OP.add
                    )
                elif eng == "A":
                    nc.scalar.activation(
                        ob[:, j], yr_sl[:, j], AF.Identity, bias=nm_ap, scale=rs_ap
                    )
                else:
                    nc.gpsimd.tensor_scalar(
                        ob[:, j], yr_sl[:, j], rs_ap, nm_ap, OP.mult, OP.add
                    )
                if affine_mode == "full":
                    nc.vector.tensor_tensor(ob[:, j], ob[:, j], g_sb, OP.mult)
                    nc.gpsimd.tensor_tensor(ob[:, j], ob[:, j], b_sb, OP.add)
            nc.sync.dma_start(out[s, :, off : off + nb], ob)

        for c in range(NC + 2):
            if c < NC:
                front(c)
            if 0 <= c - 1 < NC:
                ln_math(c - 1)
            if c >= 2:
                back(c - 2)
    nc.finalize()
    return nc


def _get_prog(affine_mode, g_u, b_u):
    key = (affine_mode, g_u, b_u)
    if key not in _prog_cache:
        _prog_cache[key] = _build(affine_mode, g_u, b_u)
    return _prog_cache[key]


def _prepare(x, W_q, W_k, W_v, W_r, mix, gamma, beta):
    import ml_dtypes

    bf16 = ml_dtypes.bfloat16
    x = np.asarray(x, dtype=np.float32)
    W_v = np.asarray(W_v, dtype=np.float32)
    W_r = np.asarray(W_r, dtype=np.float32)
    gamma = np.asarray(gamma, dtype=np.float32)
    beta = np.asarray(beta, dtype=np.float32)
    m = 1.0 / (1.0 + np.exp(-float(np.asarray(mix).reshape(-1)[0])))
    wc = np.ascontiguousarray((m * W_v + (1.0 - m) * W_r).astype(bf16))

    if np.all(gamma == gamma.flat[0]) and np.all(beta == beta.flat[0]):
        affine_mode, g_u, b_u = "none", float(gamma.flat[0]), float(beta.flat[0])
    else:
        affine_mode, g_u, b_u = "full", 1.0, 0.0

    x_flat = x.reshape(R * F, D_IN).astype(bf16)
    in_maps = []
    for c in range(N_CORES):
        shard = x_flat[c * ROWS_PER_CORE : (c + 1) * ROWS_PER_CORE]
        # [p, s, ko, r] layout: contiguous 4KB per (partition, slab)
        xt_h = np.ascontiguousarray(
            shard.reshape(N_SLABS, SLAB * P, 2, P).transpose(3, 0, 2, 1)
        )
        im = {"xt": xt_h, "w": wc}
        if affine_mode == "full":
            im["gamma"] = gamma
            im["beta"] = beta
        in_maps.append(im)
    return in_maps, affine_mode, g_u, b_u


def _unpermute_out(arr):
    # [s, p, b, n] -> rows ordered (s, b, p)
    return arr.transpose(0, 2, 1, 3).reshape(ROWS_PER_CORE, OH)


def run(trace=False, **inputs):
    """Internal entry: returns (output, BassKernelResults)."""
    from concourse.bass_utils import run_bass_kernel_spmd

    in_maps, affine_mode, g_u, b_u = _prepare(**inputs)
    nc = _get_prog(affine_mode, g_u, b_u)
    res = run_bass_kernel_spmd(nc, in_maps, core_ids=list(range(N_CORES)), trace=trace)
    parts = [
        _unpermute_out(np.asarray(r["out"], dtype=np.float32)).reshape(
            R // N_CORES, F, OH
        )
        for r in res.results
    ]
    return np.concatenate(parts, axis=0), res


def kernel(**inputs):
    out, _ = run(trace=False, **inputs)
    return out
